# revision 59
# baseline (speedup 1.0000x reference)
"""Tensor-parallel attention kernel for TRN2 (8 NeuronCores).

Sharding: 2D grid — 4 batches x 2 head-groups (8 heads each). Core c handles
batch b = c // 2, head-group g = c % 2.  Host sums the two head-group partials
per batch (the tensor-parallel "all-reduce" done in the unshard step).

Per-core schedule (single pass, PE-bound):
  1. V projection in [s, dh] layout (xT/wv stream in chunks).
  2. QK^T projection fully interleaved with attention: window w runs head
     w-1's 16 score/exp/AV slots around the projection of m-pair (w, 8+w)
     (RoPE fused at PSUM eviction), so the scalar engine's exp stream and all
     DVE work hide under tensor-engine work.
  3. Softmax denominator: running f16 adds of the exp tiles on DVE plus one
     replicated ones-matmul per head (no per-tile ones-matmuls on PE).
  4. Output projection with 4-way PSUM rotation, per-tile evict+store (f16
     partials; host sums the head-group pairs in f32).

PSUM: four [128,1024] f32 tags = all 8 banks. B0 hosts projection steps
(single-buffered — the WAR on RoPE's PSUM read hides under interleaved score
slots), B1/B2 are the score ping-pong (denominator tiles steal rotation
slots), B3 is the attention-value accumulator.
"""

from contextlib import ExitStack
from itertools import cycle

import numpy as np

B, SQ, SKV = 4, 1024, 1024
D_MODEL = 2048
N_HEADS = 16
D_HEAD = 128
ROPE_THETA = 10000.0
N_CORES = 8
HG = 8  # heads per core
P = 128

F16 = np.float16

M_ORDER = [0, 8, 1, 9, 2, 10, 3, 11, 4, 12, 5, 13, 6, 14, 7, 15]

_BUILD_CACHE = {}


def _rope_tables():
    """cosf/sinn [128, 1024] f16 for new-token positions offset + s."""
    inv_freq = 1.0 / (ROPE_THETA ** (np.arange(0, D_HEAD, 2, dtype=np.float32) / D_HEAD))
    pos = (SKV + np.arange(SQ, dtype=np.float32))
    ang = pos[:, None] * inv_freq[None, :]           # [S, 64]
    cos = np.cos(ang).astype(np.float32).T           # [64, S]
    sin = np.sin(ang).astype(np.float32).T
    cosf = np.concatenate([cos, cos], axis=0)        # [128, S]
    sinn = np.concatenate([-sin, sin], axis=0)       # [128, S]
    return (np.ascontiguousarray(cosf).astype(F16),
            np.ascontiguousarray(sinn).astype(F16))


def build_module():
    import concourse.mybir as mybir
    import concourse.tile as tile
    from concourse import bacc
    from concourse.bass import ts

    f32 = mybir.dt.float32
    f16 = mybir.dt.float16

    nc = bacc.Bacc("TRN2", target_bir_lowering=False, debug=False,
                   num_devices=N_CORES)

    d_xt = nc.dram_tensor("xt", [P, 16, SQ], f16, kind="ExternalInput").ap()
    d_wqk = nc.dram_tensor("wqk", [P, 8, 2, 16, P], f16, kind="ExternalInput").ap()
    d_wv = nc.dram_tensor("wv", [P, 16, 1024], f16, kind="ExternalInput").ap()
    d_ck = nc.dram_tensor("ck", [P, HG, SKV], f16, kind="ExternalInput").ap()
    d_cv = nc.dram_tensor("cv", [P, HG, 8, D_HEAD], f16, kind="ExternalInput").ap()
    d_wo = nc.dram_tensor("wo", [P, HG, 4, 512], f16, kind="ExternalInput").ap()
    d_cos = nc.dram_tensor("cosf", [P, SQ], f16, kind="ExternalInput").ap()
    d_sin = nc.dram_tensor("sinn", [P, SQ], f16, kind="ExternalInput").ap()
    d_out = nc.dram_tensor("out", [P, 8, 4, 512], f16, kind="ExternalOutput").ap()

    EXP = mybir.ActivationFunctionType.Exp
    MUL = mybir.AluOpType.mult
    ADD = mybir.AluOpType.add
    SCALE = float(D_HEAD) ** -0.5
    DEPTH = 5  # av lag in slots behind score/exp

    with tile.TileContext(nc) as tc, ExitStack() as ctx:
        const = ctx.enter_context(tc.tile_pool(name="const", bufs=1))
        resident = ctx.enter_context(tc.tile_pool(name="res", bufs=1))
        wqk_pool = ctx.enter_context(tc.tile_pool(name="wqk", bufs=2))
        wv_pool = ctx.enter_context(tc.tile_pool(name="wv", bufs=3))
        wo_pool = ctx.enter_context(tc.tile_pool(name="wo", bufs=2))
        tmp_pool = ctx.enter_context(tc.tile_pool(name="tmp", bufs=1))
        es_pool = ctx.enter_context(tc.tile_pool(name="exp", bufs=14))
        qk_pool = ctx.enter_context(tc.tile_pool(name="qk", bufs=6))
        ds_pool = ctx.enter_context(tc.tile_pool(name="ds", bufs=4))
        recip_pool = ctx.enter_context(tc.tile_pool(name="recip", bufs=1))
        og_pool = ctx.enter_context(tc.tile_pool(name="og", bufs=3))
        pp = ctx.enter_context(tc.tile_pool(name="pp", bufs=1, space="PSUM"))

        def ptile(tag, name):
            return pp.tile([P, SQ], f32, tag=tag, name=name)

        # ---- resident tiles ----
        xT = resident.tile([P, 16, SQ], f16, tag="xT")
        ck = resident.tile([P, HG, SKV], f16, tag="ck")
        cv = resident.tile([P, HG, 8, D_HEAD], f16, tag="cv")
        cosf = const.tile([P, SQ], f16, tag="cosf")
        sinn = const.tile([P, SQ], f16, tag="sinn")
        ones = const.tile([P, P], f16, tag="ones")
        nc.vector.memset(ones[:], 1.0)

        v_new = resident.tile([P, 2, 8, 512], f16, tag="v_new")
        attn_T = resident.tile([P, HG, SQ], f16, tag="attn_T")

        # ---- input streams (spread across three queues) ----
        # xT k-chunks: small first chunks on sync; the bulk tail rides the
        # otherwise-idle gpsimd queue, deferred clear of the critical loads
        nc.sync.dma_start(xT[:, 0, 0:512], d_xt[:, 0, 0:512])
        nc.sync.dma_start(xT[:, 0, 512:1024], d_xt[:, 0, 512:1024])
        nc.sync.dma_start(xT[:, 1:2, :], d_xt[:, 1:2, :])
        nc.scalar.dma_start(xT[:, 2:4, :], d_xt[:, 2:4, :])
        nc.sync.dma_start(xT[:, 4:8, :], d_xt[:, 4:8, :])
        nc.sync.dma_start(xT[:, 8:16, :], d_xt[:, 8:16, :])
        with tc.tile_wait_until(0.030):
            nc.sync.dma_start(cosf[:], d_cos[:])
            nc.sync.dma_start(sinn[:], d_sin[:])
        # first wqk pair-slab; later slabs prefetched one window ahead
        wqk_slabs = {}

        def wqk_load(pair):
            wt = wqk_pool.tile([P, 2, 16, P], f16, tag="wqk", name=f"wqk{pair}")
            with tc.tile_wait_until(0.034 + 0.027 * max(0, pair - 1)):
                nc.sync.dma_start(wt[:], d_wqk[:, pair, :, :, :])
            wqk_slabs[pair] = wt

        wqk_load(0)

        # ---- phase 1b: V projection [s, dh] ----
        BTAGS = ["B0", "B1", "B2", "B3"]
        for c in range(2):
            big = [ptile(BTAGS[p], f"ps_vb{p}_{c}") for p in range(4)]
            accs = [big[p][:, ts(half, 512)] for p in range(4) for half in range(2)]
            wvt = None
            wv_tiles = {}
            for k in range(16):
                if k % 4 == 0:
                    wvt = wv_pool.tile([P, 4, 512], f16, tag="wv",
                                       name=f"wv{c}_{k}")
                    wv_tiles[k // 4] = wvt
                    q = nc.scalar if c == 0 else nc.sync
                    if c == 0 and k == 0:
                        # first piece via gpsimd (wins the DMA bus early);
                        # rest streams on scalar
                        nc.gpsimd.dma_start(wvt[:, 0, :], d_wv[:, 0, ts(c, 512)])
                        q.dma_start(wvt[:, 1, :], d_wv[:, 1, ts(c, 512)])
                        q.dma_start(wvt[:, 2:4, :], d_wv[:, 2:4, ts(c, 512)])
                    else:
                        q.dma_start(wvt[:], d_wv[:, k:k + 4, ts(c, 512)])
                if c == 0 and k == 1:
                    # k0's upper-half columns land late: run (k0,k1) on the
                    # lower-half accumulators first, then both upper halves
                    for st in (2, 3, 0, 1):
                        nc.tensor.matmul(accs[st], xT[:, 1, ts(st, P)],
                                         wvt[:, 1, :], start=False, stop=False)
                    for kk in (0, 1):
                        for st in (4, 5, 6, 7):
                            nc.tensor.matmul(accs[st], xT[:, kk, ts(st, P)],
                                             wv_tiles[0][:, kk, :],
                                             start=(kk == 0), stop=False)
                    continue
                sts = (2, 3, 0, 1) if (c == 0 and k == 0)                     else (2, 3, 0, 1, 4, 5, 6, 7)
                for st in sts:
                    nc.tensor.matmul(accs[st], xT[:, k, ts(st, P)],
                                     wvt[:, k % 4, :],
                                     start=(k == 0), stop=(k == 15))
            for i, st in enumerate((2, 3, 0, 1, 4, 5, 6, 7)):
                if i % 2 == 0:
                    nc.scalar.copy(v_new[:, c, st, :], accs[st])
                else:
                    nc.vector.tensor_copy(v_new[:, c, st, :], accs[st])
        # kv cache after phase 1b's critical loads; done before the first scores
        with tc.tile_wait_until(0.036):
            nc.scalar.dma_start(ck[:], d_ck[:])
        with tc.tile_wait_until(0.044):
            nc.scalar.dma_start(cv[:], d_cv[:])

        # ---- phase 2: QK proj + RoPE interleaved with attention ----
        sc_tags = cycle(["B1", "B2"])
        es_q = {}          # slot -> es tile
        av_fifo = []       # pending slots for AV matmuls
        state = {"av": None, "ds": None, "es0": None}

        qk_tiles = {}

        def rope(m, pb):
            # qk[0:64]  = pb[0:64]*cos - pb[64:]*sin
            # qk[64:]   = pb[64:]*cos + pb[0:64]*sin
            t1 = tmp_pool.tile([P, SQ], f16, tag="t1", name=f"t1_{m}")
            t0 = tmp_pool.tile([P, SQ], f16, tag="t0", name=f"t0_{m}")
            nc.vector.tensor_tensor(t1[0:64, :], pb[64:128, :],
                                    sinn[0:64, :], MUL)
            nc.vector.tensor_tensor(t1[64:128, :], pb[0:64, :],
                                    sinn[64:128, :], MUL)
            nc.vector.tensor_tensor(t0[:], pb[:], cosf[:], MUL)
            qt = qk_pool.tile([P, SQ], f16, tag="qk", name=f"qk{m}")
            nc.vector.tensor_tensor(qt[:], t0[:], t1[:], ADD)
            qk_tiles[m] = qt

        def proj_step(m, tag):
            pair, half = M_ORDER.index(m) // 2, M_ORDER.index(m) % 2
            wt = wqk_slabs[pair]
            pb = ptile(tag, f"ps_qk{m}")
            for c in range(2):
                for k in range(16):
                    nc.tensor.matmul(pb[:, ts(c, 512)], wt[:, half, k, :],
                                     xT[:, k, ts(c, 512)],
                                     start=(k == 0), stop=(k == 15))
            rope(m, pb)

        def vtile(h, tt):
            if tt < 8:
                return cv[:, h, tt, :]
            return v_new[:, h // 4, tt - 8, ts(h % 4, P)]

        av_tiles = {}
        ds_final = {}

        def emit_av(slot):
            h, tt = slot // 16, slot % 16
            if tt == 0:
                av_tiles[h] = ptile("B3", f"ps_av{h}")
            es = es_q.pop(slot)
            for c in range(2):
                nc.tensor.matmul(av_tiles[h][:, ts(c, 512)], vtile(h, tt),
                                 es[:, ts(c, 512)],
                                 start=(tt == 0), stop=(tt == 15))

        def pump_av(lag):
            while len(av_fifo) > lag:
                emit_av(av_fifo.pop(0))

        def dn_recip_norm(h, extra=None, tag=None):
            # replicated denominator: every row of dn = sum_t es_sum[t, s]
            dn = ptile(tag or next(sc_tags), f"ps_dn{h}")
            ds = ds_final.pop(h)
            parts = [ds] if extra is None else [ds, extra]
            for c in range(2):
                for i, part in enumerate(parts):
                    nc.tensor.matmul(dn[:, ts(c, 512)], ones[:],
                                     part[:, ts(c, 512)],
                                     start=(i == 0), stop=(i == len(parts) - 1))
            rc = recip_pool.tile([P, SQ], f32, tag="rc", name=f"rc{h}")
            nc.vector.reciprocal(rc[:], dn[:])
            nc.vector.tensor_tensor(attn_T[:, h, :], av_tiles.pop(h)[:],
                                    rc[:], MUL)

        def slot(h, tt):
            s = h * 16 + tt
            kt = (ck[:, h, ts(tt, P)] if tt < 8
                  else qk_tiles[8 + h][:, ts(tt - 8, P)])
            sc = ptile(next(sc_tags), f"ps_sc{h}_{tt}")
            for c in range(2):
                nc.tensor.matmul(sc[:, ts(c, 512)], kt,
                                 qk_tiles[h][:, ts(c, 512)],
                                 start=True, stop=True)
            es = es_pool.tile([P, SQ], f16, tag="es", name=f"es{h}_{tt}")
            nc.scalar.activation(es[:], sc[:], EXP, scale=SCALE)
            es_q[s] = es
            # denominator running sum on DVE (head 7's last tile is folded
            # into the dn matmul directly to shorten the critical chain)
            if tt == 0:
                state["es0"] = es
            elif not (h == 7 and tt == 15):
                ds = ds_pool.tile([P, SQ], f16, tag="ds", name=f"ds{h}_{tt}")
                prev = state["es0"] if tt == 1 else state["ds"]
                nc.vector.tensor_tensor(ds[:], prev[:], es[:], ADD)
                state["ds"] = ds
            if tt == 15:
                ds_final[h] = state["ds"] if h < 7 else state["ds14"]
                if h == 7:
                    state["es15"] = es
            elif h == 7 and tt == 14:
                state["ds14"] = state["ds"]
            av_fifo.append(s)

        # window 0: pair (0, 8) into B1/B0 (scores haven't started, so B0
        # single-buffering and the B1 score slots never stall on RoPE reads).
        wqk_load(1)
        proj_step(0, "B1")
        # proj(8) with head-0's first two slots tucked into its tail, so the
        # exp stream (ACT) gets a head start before window 1
        _pair8, _half8 = M_ORDER.index(8) // 2, M_ORDER.index(8) % 2
        _wt8 = wqk_slabs[_pair8]
        _pb8 = ptile("B0", "ps_qk8")
        for c in range(2):
            for k in range(16):
                nc.tensor.matmul(_pb8[:, ts(c, 512)], _wt8[:, _half8, k, :],
                                 xT[:, k, ts(c, 512)],
                                 start=(k == 0), stop=(k == 15))
                if c == 1 and k == 7:
                    # tuck head-0's first slots here: RoPE(0) is done by now
                    slot(0, 0)
                    slot(0, 1)
        rope(8, _pb8)

        # Window w: head h = w-1.  Seven slots cover RoPE(8+w-1)'s PSUM reads
        # before proj(w) reuses B0; prev head's remaining AVs + its
        # denominator/normalize chain hide under those slots + proj(w); the
        # current head's first nine AVs ride slots 7-15 (exp long done).
        for w in range(1, 8):
            h = w - 1
            if w < 7:
                wqk_load(w + 1)
            else:
                # prefetch first output-projection slab instead
                wot = wo_pool.tile([P, HG, 512], f16, tag="wo", name="wo0")
                with tc.tile_wait_until(0.245):
                    nc.sync.dma_start(wot[:], d_wo[:, :, 0, :])
                wo_slabs = {0: wot}
            for tt in range(2 if w == 1 else 0, 7):
                slot(h, tt)
                pump_av(7)
            if h >= 1:
                dn_recip_norm(h - 1)
            proj_step(w, "B0")
            if w < 7:
                for tt in range(7, 16):
                    slot(h, tt)
                    pump_av(7)
                proj_step(8 + w, "B0")
            else:
                # give RoPE(15) slot cover before head 7's tt>=8 scores
                for tt in range(7, 12):
                    slot(h, tt)
                    pump_av(7)
                proj_step(15, "B0")
                for tt in range(12, 16):
                    slot(h, tt)
                    pump_av(7)
        # final head (7): no projection left to hide behind
        for tt in range(0, 7):
            slot(7, tt)
            pump_av(7)
            if tt == 0:
                wot = wo_pool.tile([P, HG, 512], f16, tag="wo", name="wo1")
                with tc.tile_wait_until(0.262):
                    nc.sync.dma_start(wot[:], d_wo[:, :, 1, :])
                wo_slabs[1] = wot
        dn_recip_norm(6, tag="B0")  # B0 idle here; avoid stealing a score tag
        for tt in range(7, 16):
            slot(7, tt)
            pump_av(max(0, 14 - tt))
        pump_av(0)
        dn_recip_norm(7, extra=state["es15"])

        # ---- phase 3: output projection (st-inner, 4-way PSUM rotation,
        # st-paired stores) ----
        acc_tags = cycle(BTAGS)
        for c4 in range(4):
            if c4 >= 2:
                wot = wo_pool.tile([P, HG, 512], f16, tag="wo", name=f"wo{c4}")
                with tc.tile_wait_until(0.272 + 0.016 * c4):
                    nc.sync.dma_start(wot[:], d_wo[:, :, c4, :])
                wo_slabs[c4] = wot
            wot = wo_slabs[c4]
            for stp in range(4):
                og = og_pool.tile([P, 2, 512], f16, tag="og",
                                  name=f"og{c4}_{stp}")
                last = c4 == 3 and stp == 3
                for half in range(2):
                    st = 2 * stp + half
                    if last and half == 1:
                        # final tile in two 256-col pieces: shorter last
                        # evict+store chain after the final matmul
                        for piece in range(2):
                            acc = pp.tile([P, SQ], f32, tag=next(acc_tags),
                                          name=f"ps_o3_7_{piece}")
                            a = acc[:, 0:256]
                            cols = slice(piece * 256, piece * 256 + 256)
                            for h in range(8):
                                nc.tensor.matmul(a, attn_T[:, h, ts(st, P)],
                                                 wot[:, h, cols],
                                                 start=(h == 0), stop=(h == 7))
                            if piece == 0:
                                nc.scalar.copy(og[:, 1, cols], a)
                            else:
                                nc.vector.tensor_copy(og[:, 1, cols], a)
                            q = nc.sync if piece == 0 else nc.scalar
                            q.dma_start(d_out[:, 7:8, 3, cols], og[:, 1:2, cols])
                        continue
                    acc = pp.tile([P, SQ], f32, tag=next(acc_tags),
                                  name=f"ps_o{c4}_{st}")
                    a = acc[:, 0:512]
                    for h in range(8):
                        nc.tensor.matmul(a, attn_T[:, h, ts(st, P)],
                                         wot[:, h, :],
                                         start=(h == 0), stop=(h == 7))
                    if half == 0:
                        nc.scalar.copy(og[:, 0, :], a)
                    else:
                        nc.vector.tensor_copy(og[:, 1, :], a)
                if last:
                    nc.sync.dma_start(d_out[:, 6:7, 3, :], og[:, 0:1, :])
                else:
                    stq = [nc.sync, nc.scalar][(c4 * 4 + stp) % 2]
                    stq.dma_start(d_out[:, 2 * stp:2 * stp + 2, c4, :], og[:])

    nc.compile()
    return nc


def _get_module():
    if "nc" not in _BUILD_CACHE:
        _BUILD_CACHE["nc"] = build_module()
    return _BUILD_CACHE["nc"]


def _prep_core_inputs(x, cache_k, cache_v, w_qkv, w_o, cosf, sinn, b, g):
    heads = list(range(g * HG, (g + 1) * HG))
    # column indices in w_qkv: head H -> q: 384H..384H+128, k: +128.., v: +256..
    qcols = np.concatenate([np.arange(384 * H, 384 * H + 128) for H in heads])
    kcols = qcols + 128
    vcols = qcols + 256

    xt = np.ascontiguousarray(x[b].T.reshape(16, P, SQ).transpose(1, 0, 2)).astype(F16)
    w_qk = w_qkv[:, np.concatenate([qcols, kcols])]             # [2048, 2048]
    # [p, m(M_ORDER), k, col]: per-m tiles contiguous per partition row
    t = w_qk.reshape(16, P, 16, P).transpose(1, 2, 0, 3)        # [p, m, k, col]
    wqk = np.ascontiguousarray(t[:, M_ORDER, :, :]).astype(F16)
    wqk = wqk.reshape(P, 8, 2, 16, P)
    w_v = w_qkv[:, vcols]                                       # [2048, 1024]
    wv = np.ascontiguousarray(w_v.reshape(16, P, 1024).transpose(1, 0, 2)).astype(F16)
    ckt = np.ascontiguousarray(cache_k[b, heads].transpose(2, 0, 1)).astype(F16)
    cvt = np.ascontiguousarray(
        cache_v[b, heads].reshape(HG, 8, P, D_HEAD).transpose(2, 0, 1, 3)).astype(F16)
    rows = np.concatenate([np.arange(P * H, P * (H + 1)) for H in heads])
    wo = np.ascontiguousarray(
        w_o[rows].reshape(HG, P, 2048).transpose(1, 0, 2)).astype(F16)
    wo = wo.reshape(P, HG, 4, 512)
    return {"xt": xt, "wqk": wqk, "wv": wv, "ck": ckt, "cv": cvt, "wo": wo,
            "cosf": cosf, "sinn": sinn}


def kernel(x, cache_k, cache_v, w_qkv, w_o, trace=False):
    from concourse import bass_utils

    nc = _get_module()
    cosf, sinn = _rope_tables()
    x = np.asarray(x); cache_k = np.asarray(cache_k); cache_v = np.asarray(cache_v)
    w_qkv = np.asarray(w_qkv); w_o = np.asarray(w_o)

    in_maps = []
    for core in range(N_CORES):
        b, g = core // 2, core % 2
        in_maps.append(_prep_core_inputs(x, cache_k, cache_v, w_qkv, w_o,
                                         cosf, sinn, b, g))

    res = bass_utils.run_bass_kernel_spmd(nc, in_maps,
                                          core_ids=list(range(N_CORES)),
                                          trace=trace)
    _BUILD_CACHE["last_result"] = res
    out = np.zeros((B, SQ, D_MODEL), dtype=np.float32)
    for core in range(N_CORES):
        o = res.results[core]["out"].astype(np.float32)      # [P, 8, 4, 512]
        out[core // 2] += o.transpose(1, 0, 2, 3).reshape(SQ, D_MODEL)
    return out


if __name__ == "__main__":
    rng = np.random.default_rng(0)
    ins = {
        "x": rng.standard_normal((B, SQ, D_MODEL), dtype=np.float32),
        "cache_k": rng.standard_normal((B, N_HEADS, SKV, D_HEAD), dtype=np.float32),
        "cache_v": rng.standard_normal((B, N_HEADS, SKV, D_HEAD), dtype=np.float32),
        "w_qkv": rng.standard_normal((D_MODEL, 3 * D_MODEL), dtype=np.float32) * D_MODEL ** -0.5,
        "w_o": rng.standard_normal((D_MODEL, D_MODEL), dtype=np.float32) * D_MODEL ** -0.5,
    }
    out = kernel(**ins)
    print("out", out.shape, out.dtype, float(np.abs(out).max()))
